# revision 25
# baseline (speedup 1.0000x reference)
"""GCN message-passing kernel for 8 Trainium2 NeuronCores.

Strategy: shard CHANNELS across the 8 cores (C=1280 -> 160 ch/core). Each core
computes the full output for its channel slice with zero collectives:
  - edge-encoder MLP: gamma/beta only for this core's 160 channels (W2 column
    shard), with the full h = relu(pose @ W1 + b1) recomputed per core on PE
    using 2x row-tiling (K=9 fits a 32-row group; two k-tiles run concurrently).
  - edges are host-sorted by dst and padded per 128-node dst window; the
    scatter-sum is a one-hot matmul on PE accumulating into PSUM per window.
    Blocks are processed in PAIRS with fp8 DoubleRow (K=256) so the one-hot
    weight load amortizes over 6 matmuls and the m stream halves.
  - the per-edge gather image[src] is an indirect DMA from an HBM-resident
    per-core image slice laid out [node, hw, ch] in bf16.
  - mean = PSUM evacuation with per-partition scale 1/cnt (0 for empty nodes).
"""

import sys

sys.path.insert(0, "/opt/trn_rl_repo")

import numpy as np
import ml_dtypes

import concourse.bass as bass
import concourse.mybir as mybir
from concourse.tile import TileContext
from concourse.bass_utils import run_bass_kernel_spmd

BF16 = ml_dtypes.bfloat16
FP8 = ml_dtypes.float8_e4m3
P = 128
N_CORES = 8
CH_EDGES = 1024  # edges per h-chunk
BPC = CH_EDGES // P  # blocks per chunk

ET_MODE = "dr"  # eT matmul: "nodr" (fp8+FWL) | "dr" (DoubleRow)
SC_MODE = "bf16"  # scatter: "dr" (fp8 DoubleRow pairs) | "bf16"
SC_DELAY = 2  # pairs of scatter-emission delay (software pipelining)
HT_DVE_RATIO = 4  # every HT_DVE_RATIO'th hT evacuation goes to DVE, rest ACT


def _split_excess_waits(nc):
    """This walrus build only encodes 1 sem-wait per instruction; hoist extra
    waits onto same-engine NoOps placed just before (engines run in order)."""
    for bb in nc.main_func.blocks:
        new_insts = []
        for ins in bb.instructions:
            si = ins.sync_info
            limit = 1
            if si is not None and si.on_wait and len(si.on_wait) > limit:
                waits = list(si.on_wait)
                extra, keep = waits[:-limit], waits[-limit:]
                for k, w in enumerate(extra):
                    nop = mybir.InstNoOp(name=f"{ins.name}-ws-{k}", ins=[], outs=[])
                    nop.engine = ins.engine
                    nop.sync_info = mybir.SyncInfo(on_wait=[w], on_update=[])
                    new_insts.append(nop)
                si.on_wait = keep
            new_insts.append(ins)
        bb.instructions[:] = new_insts


def _host_prep(pose, image, W1, b1, W2, b2, src, dst):
    """Sort/pad edges by dst window, build per-core shards and onehot pairs."""
    E = pose.shape[0]
    Nn, C, H, Wsp = image.shape
    HW = H * Wsp
    CS = C // N_CORES
    F = CS * HW
    n_win = Nn // P

    src = np.asarray(src).astype(np.int64)
    dst = np.asarray(dst).astype(np.int64)

    order = np.argsort(dst, kind="stable")
    blk_edge = []  # [B, 128] edge id, -1 = pad
    blk_win = []
    for w in range(n_win):
        sel = order[(dst[order] >= w * P) & (dst[order] < (w + 1) * P)]
        nb = max(1, -(-len(sel) // P))
        if nb % 2:  # even block count per window -> clean DR pairs
            nb += 1
        for b in range(nb):
            seg = sel[b * P : (b + 1) * P]
            row = np.full(P, -1, np.int64)
            row[: len(seg)] = seg
            blk_edge.append(row)
            blk_win.append(w)
    blk_edge = np.stack(blk_edge)  # [B, 128]
    B = len(blk_edge)

    valid = blk_edge >= 0
    eids = np.where(valid, blk_edge, 0)

    # gather src per edge slot (pad -> node 0)
    blk_src = np.where(valid, src[eids], 0).astype(np.int32)  # [B,128]
    # onehot: [B, 128 edge, 128 local-node], zero row for pads
    loc = (np.where(valid, dst[eids], 0) - np.asarray(blk_win)[:, None] * P).astype(
        np.int64
    )
    oh = np.zeros((B, P, P), np.float32)
    bi, pi = np.nonzero(valid)
    oh[bi, pi, loc[bi, pi]] = 1.0

    # pairs (within window; B even per window by construction)
    pair_blk = []  # [NP, 2]
    pair_win = []
    b = 0
    while b < B:
        assert blk_win[b] == blk_win[b + 1]
        pair_blk.append((b, b + 1))
        pair_win.append(blk_win[b])
        b += 2
    NP = len(pair_blk)
    first_pair = {}
    last_pair = {}
    for pi_, w in enumerate(pair_win):
        first_pair.setdefault(w, pi_)
        last_pair[w] = pi_

    # edge chunks for h: pad edge count to CH_EDGES multiple
    Ep = -(-B * P // CH_EDGES) * CH_EDGES
    NCH = Ep // CH_EDGES

    # poseT padded: [9, Ep]
    pose_pad = np.zeros((Ep, 9), np.float32)
    pose_pad[: B * P] = np.where(valid.reshape(-1, 1), pose[eids.reshape(-1)], 0.0)
    poseT = np.ascontiguousarray(pose_pad.T.astype(BF16))  # [9, Ep]

    # oh, laid out for the scatter mode
    if SC_MODE == "dr":
        # per pair: [128 slot, 2*128]: [:, j*128+n] = oh[blk_j][slot, n], fp8
        oh_pairs = np.zeros((P, NP * 2 * P), np.float32)
        for pi_, (ba, bb) in enumerate(pair_blk):
            oh_pairs[:, pi_ * 2 * P : pi_ * 2 * P + P] = oh[ba]
            oh_pairs[:, pi_ * 2 * P + P : (pi_ + 1) * 2 * P] = oh[bb]
        oh_host = oh_pairs.astype(FP8)  # [128, NP*256]
    else:
        oh_host = (
            oh.transpose(1, 0, 2).reshape(P, B * P).astype(BF16)
        )  # [128, B*128]

    b2_allzero = not np.any(b2)
    b1_allzero = not np.any(b1)
    cnt = np.bincount(dst, minlength=Nn).astype(np.float32)
    recip = np.where(cnt > 0, 1.0 / np.maximum(cnt, 1.0), 0.0).astype(np.float32)
    recip_t = np.ascontiguousarray(recip.reshape(n_win, P).T)  # [P, n_win]

    KT = C // P
    b1t = np.ascontiguousarray(b1.astype(np.float32).reshape(KT, P).T)  # [P, KT]
    idx_t = np.ascontiguousarray(blk_src.T)  # [P, B]

    shared = dict(
        poseT=poseT,
        w1=W1.astype(BF16),
        b1t=b1t,
        idx=idx_t,
        oh=oh_host,
        recip=recip_t,
    )
    in_maps = []
    for j in range(N_CORES):
        c0 = j * CS
        cols_g = [2 * (c0 + i) for i in range(CS)]
        cols_b = [2 * (c0 + i) + 1 for i in range(CS)]
        cols = cols_g + cols_b
        w2f8 = W2[:, cols].astype(FP8)  # [C, 2*CS]
        b2row = b2[cols].reshape(1, -1).astype(BF16)  # [1, 2*CS]
        img = (
            image[:, c0 : c0 + CS]
            .transpose(0, 2, 3, 1)
            .reshape(Nn, F)
            .astype(BF16)
        )  # [Nn, F] layout [n, hw, c]
        in_maps.append(dict(shared, w2f8=w2f8, b2row=b2row, image=img))

    meta = dict(
        E=E, Nn=Nn, C=C, HW=HW, CS=CS, F=F, n_win=n_win, B=B, Ep=Ep, KT=KT,
        NP=NP, NCH=NCH, pair_blk=pair_blk, pair_win=pair_win,
        first_pair=first_pair, last_pair=last_pair, b2_allzero=b2_allzero,
        b1_allzero=b1_allzero,
    )
    return in_maps, meta


def _build(meta):
    Nn, CS, F, HW = meta["Nn"], meta["CS"], meta["F"], meta["HW"]
    n_win, B, Ep, KT = meta["n_win"], meta["B"], meta["Ep"], meta["KT"]
    C, NP, NCH = meta["C"], meta["NP"], meta["NCH"]
    pair_blk, pair_win = meta["pair_blk"], meta["pair_win"]
    first_pair, last_pair = meta["first_pair"], meta["last_pair"]
    f32 = mybir.dt.float32
    bf16 = mybir.dt.bfloat16
    fp8 = mybir.dt.float8e4
    i32 = mybir.dt.int32
    FS = F + CS  # scatter width: gamma*x features + beta column block
    seg_cols = [(s, min(512, F - s)) for s in range(0, F, 512)]
    m_dt = fp8 if SC_MODE == "dr" else bf16
    oh_dt = fp8 if SC_MODE == "dr" else bf16
    oh_w = NP * 2 * P if SC_MODE == "dr" else B * P

    nc = bass.Bass()
    poseT_d = nc.declare_dram_parameter("poseT", [9, Ep], bf16, isOutput=False)
    w1_d = nc.declare_dram_parameter("w1", [9, C], bf16, isOutput=False)
    b1t_d = nc.declare_dram_parameter("b1t", [P, KT], f32, isOutput=False)
    w2f8_d = nc.declare_dram_parameter("w2f8", [C, 2 * CS], fp8, isOutput=False)
    b2_d = nc.declare_dram_parameter("b2row", [1, 2 * CS], bf16, isOutput=False)
    img_d = nc.declare_dram_parameter("image", [Nn, F], bf16, isOutput=False)
    idx_d = nc.declare_dram_parameter("idx", [P, B], i32, isOutput=False)
    oh_d = nc.declare_dram_parameter("oh", [P, oh_w], oh_dt, isOutput=False)
    recip_d = nc.declare_dram_parameter("recip", [P, n_win], f32, isOutput=False)
    out_d = nc.declare_dram_parameter("out", [Nn, F], bf16, isOutput=True)

    with TileContext(nc) as tc:
        with (
            tc.tile_pool(name="const", bufs=1) as constp,
            tc.tile_pool(name="ht", bufs=3) as htp,
            tc.tile_pool(name="gb", bufs=6) as gbp,
            tc.tile_pool(name="xg", bufs=6) as xp,
            tc.tile_pool(name="mm", bufs=4) as mp,
            tc.tile_pool(name="outp", bufs=2) as outp,
            tc.tile_pool(name="bsm", bufs=2) as bsmp,
            tc.tile_pool(name="pw", bufs=1, space="PSUM") as pwp,
            tc.tile_pool(name="ps", bufs=3, space="PSUM") as psp,
        ):
            # ---- preload constants (first chunk's pose slice first, so the
            # PE can start immediately; the rest streams behind it) ----
            w1_sb = constp.tile([41, C], bf16)
            nc.sync.dma_start(out=w1_sb[0:9, :], in_=w1_d[:])
            nc.sync.dma_start(out=w1_sb[32:41, :], in_=w1_d[:])
            poseT_sb = constp.tile([41, Ep], bf16)
            nc.scalar.dma_start(
                out=poseT_sb[0:9, :CH_EDGES], in_=poseT_d[:, :CH_EDGES]
            )
            nc.scalar.dma_start(
                out=poseT_sb[32:41, :CH_EDGES], in_=poseT_d[:, :CH_EDGES]
            )
            nc.sync.dma_start(out=poseT_sb[0:9, CH_EDGES:], in_=poseT_d[:, CH_EDGES:])
            nc.sync.dma_start(
                out=poseT_sb[32:41, CH_EDGES:], in_=poseT_d[:, CH_EDGES:]
            )
            b1_sb = constp.tile([P, KT], f32)
            nc.sync.dma_start(out=b1_sb[:], in_=b1t_d[:])
            w2_sb = constp.tile([P, KT * 2 * CS], fp8)
            for t in range(KT):
                nc.sync.dma_start(
                    out=w2_sb[:, t * 2 * CS : (t + 1) * 2 * CS],
                    in_=w2f8_d[t * P : (t + 1) * P, :],
                )
            b2_sb = constp.tile([1, 2 * CS], bf16)
            if not meta["b2_allzero"]:
                nc.sync.dma_start(out=b2_sb[:], in_=b2_d[:])
            idx_sb = constp.tile([P, B], i32)
            nc.sync.dma_start(out=idx_sb[:], in_=idx_d[:])
            oh_sb = constp.tile([P, oh_w], oh_dt)
            nc.sync.dma_start(out=oh_sb[:], in_=oh_d[:])
            recip_sb = constp.tile([P, n_win], f32)
            nc.sync.dma_start(out=recip_sb[:], in_=recip_d[:])
            ones_sb = constp.tile([1, P], bf16)
            if not meta["b2_allzero"]:
                nc.gpsimd.memset(ones_sb[:], 1.0)

            w23 = w2_sb.rearrange("p (t c) -> p t c", t=KT)

            hT_tiles = {}  # chunk -> tile
            h_queue = []  # pending (ci, g, e0) h iterations, FIFO
            h_queued_ci = -1  # last chunk whose iterations were enqueued
            evac_ctr = [0]

            def queue_h(ci):
                nonlocal h_queued_ci
                while h_queued_ci < ci:
                    h_queued_ci += 1
                    if h_queued_ci >= NCH:
                        return
                    hT_tiles[h_queued_ci] = htp.tile(
                        [P, KT * CH_EDGES], fp8, tag="ht", name=f"hT{h_queued_ci}"
                    )
                    for g in range(KT // 2):
                        for e0 in range(0, CH_EDGES, 512):
                            h_queue.append((h_queued_ci, g, e0))

            def emit_h_iter(ci, g, e0):
                """one 2x row-tiled K=9 matmul pair of the hT chunk:
                hT = relu(W1.T posT + b1), stored fp8."""
                hT = hT_tiles[ci]
                e_lo = ci * CH_EDGES
                pha = psp.tile([P, 512], f32, tag="ps")
                phb = psp.tile([P, 512], f32, tag="ps")
                t0, t1 = 2 * g, 2 * g + 1
                nc.tensor.matmul(
                    out=pha[:],
                    lhsT=w1_sb[0:9, t0 * P : (t0 + 1) * P],
                    rhs=poseT_sb[0:9, e_lo + e0 : e_lo + e0 + 512],
                    start=True,
                    stop=True,
                    tile_position=(0, 0),
                )
                nc.tensor.matmul(
                    out=phb[:],
                    lhsT=w1_sb[32:41, t1 * P : (t1 + 1) * P],
                    rhs=poseT_sb[32:41, e_lo + e0 : e_lo + e0 + 512],
                    start=True,
                    stop=True,
                    tile_position=(32, 0),
                )
                for t, ph in ((t0, pha), (t1, phb)):
                    dstv = hT[:, t * CH_EDGES + e0 : t * CH_EDGES + e0 + 512]
                    # alternate PSUM evacuation between ACT and DVE
                    evac_ctr[0] += 1
                    if evac_ctr[0] % HT_DVE_RATIO != HT_DVE_RATIO - 1:
                        nc.scalar.activation(
                            dstv,
                            ph[:],
                            mybir.ActivationFunctionType.Relu,
                            bias=b1_sb[:, t : t + 1],
                            scale=1.0,
                        )
                    elif meta["b1_allzero"]:
                        nc.vector.tensor_scalar_max(out=dstv, in0=ph[:], scalar1=0.0)
                    else:
                        nc.vector.tensor_scalar(
                            out=dstv,
                            in0=ph[:],
                            scalar1=b1_sb[:, t : t + 1],
                            scalar2=0.0,
                            op0=mybir.AluOpType.add,
                            op1=mybir.AluOpType.max,
                        )

            def pump_h(n):
                for _ in range(min(n, len(h_queue))):
                    emit_h_iter(*h_queue.pop(0))

            def flush_h(ci):
                """emit all pending h work for chunks <= ci"""
                queue_h(ci)
                while h_queue and h_queue[0][0] <= ci:
                    emit_h_iter(*h_queue.pop(0))

            def emit_gb(b, hT, ci):
                """gamma/beta for one 128-edge block -> bf16 [128, 2CS] half."""
                bi = b - ci * BPC
                pe_ps = psp.tile([P, 512], f32, tag="ps")
                if ET_MODE == "dr":
                    hT3 = hT.rearrange("p (t e) -> p t e", t=KT)
                    for t2 in range(KT // 2):
                        nc.tensor.matmul(
                            out=pe_ps[:, : 2 * CS],
                            lhsT=hT3[:, 2 * t2 : 2 * t2 + 2, bi * P : (bi + 1) * P],
                            rhs=w23[:, 2 * t2 : 2 * t2 + 2, :],
                            start=(t2 == 0),
                            stop=(t2 == KT // 2 - 1 and meta["b2_allzero"]),
                            perf_mode=mybir.MatmulPerfMode.DoubleRow,
                        )
                else:  # nodr: plain fp8, FWL weight loads
                    for t in range(KT):
                        nc.tensor.matmul(
                            out=pe_ps[:, : 2 * CS],
                            lhsT=hT[
                                :, t * CH_EDGES + bi * P : t * CH_EDGES + (bi + 1) * P
                            ],
                            rhs=w23[:, t, :],
                            start=(t == 0),
                            stop=(t == KT - 1 and meta["b2_allzero"]),
                        )
                if not meta["b2_allzero"]:
                    nc.tensor.matmul(
                        out=pe_ps[:, : 2 * CS],
                        lhsT=ones_sb[:1, :P],
                        rhs=b2_sb[:1, :],
                        start=False,
                        stop=True,
                    )
                return pe_ps

            GA = 5  # pairs of gather lookahead

            def emit_gather(pi_):
                ba, bb = pair_blk[pi_]
                X2 = xp.tile([P, 2 * F], bf16, tag="xg", name=f"X2_{pi_}")
                for j, b in enumerate((ba, bb)):
                    nc.gpsimd.indirect_dma_start(
                        out=X2[:, j * F : (j + 1) * F],
                        out_offset=None,
                        in_=img_d[:],
                        in_offset=bass.IndirectOffsetOnAxis(
                            ap=idx_sb[:, b : b + 1], axis=0
                        ),
                    )
                return X2

            X2_by_pair = {}
            for pi_ in range(min(GA, NP)):
                X2_by_pair[pi_] = emit_gather(pi_)

            # ---- HAM warm-up: ~24 dense matmuls into one scratch PSUM bank
            # (start=True each: pure overwrite, no evacuation, no pool waits)
            # so the PE clock reaches 8/8 before the real pipeline begins.
            warm_ps = psp.tile([P, 512], f32, tag="ps")
            for _ in range(24):
                nc.tensor.matmul(
                    out=warm_ps[:],
                    lhsT=w1_sb[0:9, 0:P],
                    rhs=poseT_sb[0:9, 0:512],
                    start=True,
                    stop=True,
                )
            # ---- main pipeline over pairs ----
            # The scatter for pair p is emitted SC_DELAY pairs late so the PE
            # never waits on DVE's m-multiply: by the time scatter_p enters the
            # PE queue, m2_p has had ~2 pair-times to complete.
            psw_by_win = {}
            pending = []  # delayed scatter closures

            def emit_scatter(pi_, ba, bb, w, m2, gbm2):
                first = first_pair[w] == pi_
                last = last_pair[w] == pi_
                if first:
                    psw_by_win[w] = (
                        pwp.tile([P, F], f32, tag="pw", name=f"psw{w}"),
                        bsmp.tile([P, CS], f32, tag="bsm", name=f"bsum{w}"),
                    )
                psw_l, bsum_l = psw_by_win[w]

                if SC_MODE == "dr":
                    oh2 = oh_sb[:, pi_ * 2 * P : (pi_ + 1) * 2 * P].rearrange(
                        "p (j n) -> p j n", j=2
                    )
                    m3 = m2.rearrange("p (j f) -> p j f", j=2)
                    mb3 = gbm2.rearrange("p (j c) -> p j c", j=2)[:, :, CS:]
                    for s, width in seg_cols:
                        nc.tensor.matmul(
                            out=psw_l[:, s : s + width],
                            lhsT=oh2,
                            rhs=m3[:, :, s : s + width],
                            start=first,
                            stop=last,
                            perf_mode=mybir.MatmulPerfMode.DoubleRow,
                            skip_group_check=True,
                        )
                    pb_t = psp.tile([P, 512], f32, tag="ps", name=f"pbd{pi_}")
                    nc.tensor.matmul(
                        out=pb_t[:, :CS],
                        lhsT=oh2,
                        rhs=mb3,
                        start=True,
                        stop=True,
                        perf_mode=mybir.MatmulPerfMode.DoubleRow,
                    )
                    if first:
                        nc.vector.tensor_copy(out=bsum_l[:], in_=pb_t[:, :CS])
                    else:
                        nc.vector.tensor_add(
                            out=bsum_l[:], in0=bsum_l[:], in1=pb_t[:, :CS]
                        )
                else:
                    for j, b in enumerate((ba, bb)):
                        oht = oh_sb[:, b * P : (b + 1) * P]
                        for s, width in seg_cols:
                            nc.tensor.matmul(
                                out=psw_l[:, s : s + width],
                                lhsT=oht,
                                rhs=m2[:, j * F + s : j * F + s + width],
                                start=first and j == 0,
                                stop=last and j == 1,
                                skip_group_check=True,
                            )
                        pb_t = psp.tile([P, 512], f32, tag="ps", name=f"pb{pi_}_{j}")
                        nc.tensor.matmul(
                            out=pb_t[:, :CS],
                            lhsT=oht,
                            rhs=gbm2[:, j * 2 * CS + CS : (j + 1) * 2 * CS],
                            start=True,
                            stop=True,
                        )
                        if first and j == 0:
                            nc.vector.tensor_copy(out=bsum_l[:], in_=pb_t[:, :CS])
                        else:
                            nc.vector.tensor_add(
                                out=bsum_l[:], in0=bsum_l[:], in1=pb_t[:, :CS]
                            )

                if last:
                    # ---- evacuate window: out = psw*recip + (beta_seg*recip)
                    # split by column halves across ACT and DVE so the PSUM
                    # window frees ~2x sooner (it gates the next window)
                    bs = outp.tile([P, CS], bf16, tag="bs")
                    nc.scalar.activation(
                        bs[:],
                        bsum_l[:],
                        mybir.ActivationFunctionType.Copy,
                        scale=recip_sb[:, w : w + 1],
                    )
                    HF = F // 2
                    HO = HW // 2
                    bs_b = bs.rearrange("p (o c) -> p o c", o=1)
                    of = outp.tile([P, F], bf16, tag="of")
                    of3 = of.rearrange("p (o c) -> p o c", o=HW)
                    psw3 = psw_l[:, :F].rearrange("p (o c) -> p o c", o=HW)
                    om = outp.tile([P, HF], bf16, tag="om")
                    nc.scalar.activation(
                        om[:],
                        psw_l[:, :HF],
                        mybir.ActivationFunctionType.Copy,
                        scale=recip_sb[:, w : w + 1],
                    )
                    nc.vector.scalar_tensor_tensor(
                        out=of3[:, HO:, :],
                        in0=psw3[:, HO:, :],
                        scalar=recip_sb[:, w : w + 1],
                        in1=bs_b.to_broadcast([P, HO, CS]),
                        op0=mybir.AluOpType.mult,
                        op1=mybir.AluOpType.add,
                    )
                    nc.vector.tensor_tensor(
                        out=of3[:, :HO, :],
                        in0=om.rearrange("p (o c) -> p o c", o=HO),
                        in1=bs_b.to_broadcast([P, HO, CS]),
                        op=mybir.AluOpType.add,
                    )
                    nc.sync.dma_start(out=out_d[w * P : (w + 1) * P, :], in_=of[:])

            for pi_ in range(NP):
                ba, bb = pair_blk[pi_]
                w = pair_win[pi_]
                ci_need = bb // BPC
                flush_h(ci_need)  # h this pair depends on: emit now
                queue_h(ci_need + 1)  # next chunk's h: trickle between pairs

                # ---- gather X: prefetched GA pairs ahead
                if pi_ + GA < NP:
                    X2_by_pair[pi_ + GA] = emit_gather(pi_ + GA)
                X2 = X2_by_pair.pop(pi_)

                # ---- gamma (bf16, feeds DVE) and beta (m_dt) for both blocks
                gbm2 = gbp.tile([P, 2 * 2 * CS], bf16, tag="gb")
                m2 = mp.tile([P, 2 * F], m_dt, tag="mm")
                for j, b in enumerate((ba, bb)):
                    pe_ps = emit_gb(b, hT_tiles[b // BPC], b // BPC)
                    nc.scalar.activation(
                        gbm2[:, j * 2 * CS : (j + 1) * 2 * CS],
                        pe_ps[:, : 2 * CS],
                        mybir.ActivationFunctionType.Sigmoid,
                    )
                    pump_h(1)  # keep PE fed while sigmoid evacuates pe_ps

                # ---- m = gamma (bcast over hw) * X, per block
                for j in range(2):
                    g_b = (
                        gbm2[:, j * 2 * CS : j * 2 * CS + CS]
                        .rearrange("p (o c) -> p o c", o=1)
                        .to_broadcast([P, HW, CS])
                    )
                    nc.vector.tensor_tensor(
                        out=m2[:, j * F : (j + 1) * F].rearrange(
                            "p (o c) -> p o c", o=HW
                        ),
                        in0=X2[:, j * F : (j + 1) * F].rearrange(
                            "p (o c) -> p o c", o=HW
                        ),
                        in1=g_b,
                        op=mybir.AluOpType.mult,
                    )

                pending.append((pi_, ba, bb, w, m2, gbm2))
                if len(pending) > SC_DELAY:
                    emit_scatter(*pending.pop(0))
                pump_h(1)

            while pending:
                emit_scatter(*pending.pop(0))

    _split_excess_waits(nc)
    return nc


def _run(inputs, trace=False, trace_kwargs=None):
    pose = np.asarray(inputs["pose"], np.float32)
    image = np.asarray(inputs["image"], np.float32)
    W1 = np.asarray(inputs["W1"], np.float32)
    b1 = np.asarray(inputs["b1"], np.float32)
    W2 = np.asarray(inputs["W2"], np.float32)
    b2 = np.asarray(inputs["b2"], np.float32)
    src = np.asarray(inputs["src"])
    dst = np.asarray(inputs["dst"])

    in_maps, meta = _host_prep(pose, image, W1, b1, W2, b2, src, dst)
    nc = _build(meta)
    kw = {}
    if trace:
        kw = dict(trace=True, trace_kwargs=trace_kwargs or {})
    res = run_bass_kernel_spmd(nc, in_maps, core_ids=list(range(N_CORES)), **kw)
    Nn, C, HW, CS = meta["Nn"], meta["C"], meta["HW"], meta["CS"]
    H = int(np.sqrt(HW))
    out = np.empty((Nn, C, H, HW // H), np.float32)
    for j in range(N_CORES):
        oc = np.asarray(res.results[j]["out"]).astype(np.float32)
        out[:, j * CS : (j + 1) * CS] = (
            oc.reshape(Nn, HW, CS).transpose(0, 2, 1).reshape(Nn, CS, H, HW // H)
        )
    return out, res


def kernel(**inputs) -> np.ndarray:
    out, _ = _run(inputs)
    return out


# revision 26
# speedup vs baseline: 1.2373x; 1.2373x over previous
"""GCN message-passing kernel for 8 Trainium2 NeuronCores.

Strategy: shard CHANNELS across the 8 cores (C=1280 -> 160 ch/core). Each core
computes the full output for its channel slice with zero collectives:
  - edge-encoder MLP: gamma/beta only for this core's 160 channels (W2 column
    shard), with the full h = relu(pose @ W1 + b1) recomputed per core on PE
    using 2x row-tiled K=9 matmuls, trickled between blocks to keep PE dense.
  - edges are host-sorted by dst and padded per 128-node dst window; the
    scatter-sum is a one-hot matmul on PE. The scatter runs SEG-MAJOR: each
    512-col segment of a window accumulates in its own single-bank PSUM tile
    during a dense per-window burst, so the scatter holds only ~3 PSUM banks
    and the h/eT pipeline gets a deep (5-buffer) PSUM pool -- the PE almost
    never waits on ACT/DVE evacuations.
  - the per-edge gather image[src] is an indirect DMA from an HBM-resident
    per-core image slice laid out [node, hw, ch] in bf16.
  - mean = PSUM evacuation with per-partition scale 1/cnt (0 for empty nodes);
    the per-window beta sums are added in one broadcast DVE pass at the end.
"""

import sys

sys.path.insert(0, "/opt/trn_rl_repo")

import numpy as np
import ml_dtypes

import concourse.bass as bass
import concourse.mybir as mybir
from concourse.tile import TileContext
from concourse.bass_utils import run_bass_kernel_spmd

BF16 = ml_dtypes.bfloat16
P = 128
N_CORES = 8
CH_EDGES = 1024  # edges per h-chunk
BPC = CH_EDGES // P  # blocks per chunk
HT_DVE_RATIO = 4  # every HT_DVE_RATIO'th hT evacuation goes to DVE, rest ACT
GA = 6  # blocks of gather lookahead


def _split_excess_waits(nc):
    """This walrus build only encodes 1 sem-wait per instruction; hoist extra
    waits onto same-engine NoOps placed just before (engines run in order)."""
    for bb in nc.main_func.blocks:
        new_insts = []
        for ins in bb.instructions:
            si = ins.sync_info
            limit = 1
            if si is not None and si.on_wait and len(si.on_wait) > limit:
                waits = list(si.on_wait)
                extra, keep = waits[:-limit], waits[-limit:]
                for k, w in enumerate(extra):
                    nop = mybir.InstNoOp(name=f"{ins.name}-ws-{k}", ins=[], outs=[])
                    nop.engine = ins.engine
                    nop.sync_info = mybir.SyncInfo(on_wait=[w], on_update=[])
                    new_insts.append(nop)
                si.on_wait = keep
            new_insts.append(ins)
        bb.instructions[:] = new_insts


def _host_prep(pose, image, W1, b1, W2, b2, src, dst):
    """Sort/pad edges by dst window, build per-core shards and onehot blocks."""
    E = pose.shape[0]
    Nn, C, H, Wsp = image.shape
    HW = H * Wsp
    CS = C // N_CORES
    F = CS * HW
    n_win = Nn // P

    src = np.asarray(src).astype(np.int64)
    dst = np.asarray(dst).astype(np.int64)

    order = np.argsort(dst, kind="stable")
    blk_edge = []  # [B, 128] edge id, -1 = pad
    blk_win = []
    for w in range(n_win):
        sel = order[(dst[order] >= w * P) & (dst[order] < (w + 1) * P)]
        nb = max(1, -(-len(sel) // P))
        for b in range(nb):
            seg = sel[b * P : (b + 1) * P]
            row = np.full(P, -1, np.int64)
            row[: len(seg)] = seg
            blk_edge.append(row)
            blk_win.append(w)
    blk_edge = np.stack(blk_edge)  # [B, 128]
    B = len(blk_edge)

    valid = blk_edge >= 0
    eids = np.where(valid, blk_edge, 0)

    blk_src = np.where(valid, src[eids], 0).astype(np.int32)  # [B,128]
    loc = (np.where(valid, dst[eids], 0) - np.asarray(blk_win)[:, None] * P).astype(
        np.int64
    )
    oh = np.zeros((B, P, P), np.float32)
    bi, pi = np.nonzero(valid)
    oh[bi, pi, loc[bi, pi]] = 1.0
    oh_host = oh.transpose(1, 0, 2).reshape(P, B * P).astype(BF16)  # [128, B*128]

    win_blocks = {w: [] for w in range(n_win)}
    for b, w in enumerate(blk_win):
        win_blocks[w].append(b)

    # edge chunks for h
    Ep = -(-B * P // CH_EDGES) * CH_EDGES
    NCH = Ep // CH_EDGES

    pose_pad = np.zeros((Ep, 9), np.float32)
    pose_pad[: B * P] = np.where(valid.reshape(-1, 1), pose[eids.reshape(-1)], 0.0)
    poseT = np.ascontiguousarray(pose_pad.T.astype(BF16))  # [9, Ep]

    b2_allzero = not np.any(b2)
    b1_allzero = not np.any(b1)
    cnt = np.bincount(dst, minlength=Nn).astype(np.float32)
    recip = np.where(cnt > 0, 1.0 / np.maximum(cnt, 1.0), 0.0).astype(np.float32)
    recip_t = np.ascontiguousarray(recip.reshape(n_win, P).T)  # [P, n_win]

    KT = C // P
    b1t = np.ascontiguousarray(b1.astype(np.float32).reshape(KT, P).T)  # [P, KT]
    idx_t = np.ascontiguousarray(blk_src.T)  # [P, B]

    shared = dict(
        poseT=poseT,
        w1=W1.astype(BF16),
        b1t=b1t,
        idx=idx_t,
        oh=oh_host,
        recip=recip_t,
    )
    in_maps = []
    for j in range(N_CORES):
        c0 = j * CS
        cols_g = [2 * (c0 + i) for i in range(CS)]
        cols_b = [2 * (c0 + i) + 1 for i in range(CS)]
        cols = cols_g + cols_b
        w2f8 = W2[:, cols].astype(ml_dtypes.float8_e4m3)  # [C, 2*CS]
        b2row = b2[cols].reshape(1, -1).astype(BF16)  # [1, 2*CS]
        img = (
            image[:, c0 : c0 + CS]
            .transpose(0, 2, 3, 1)
            .reshape(Nn, F)
            .astype(BF16)
        )  # [Nn, F] layout [n, hw, c]
        in_maps.append(dict(shared, w2f8=w2f8, b2row=b2row, image=img))

    meta = dict(
        E=E, Nn=Nn, C=C, HW=HW, CS=CS, F=F, n_win=n_win, B=B, Ep=Ep, KT=KT,
        NCH=NCH, blk_win=blk_win, win_blocks=win_blocks,
        b2_allzero=b2_allzero, b1_allzero=b1_allzero,
    )
    return in_maps, meta


def _build(meta):
    Nn, CS, F, HW = meta["Nn"], meta["CS"], meta["F"], meta["HW"]
    n_win, B, Ep, KT = meta["n_win"], meta["B"], meta["Ep"], meta["KT"]
    C, NCH = meta["C"], meta["NCH"]
    win_blocks = meta["win_blocks"]
    f32 = mybir.dt.float32
    bf16 = mybir.dt.bfloat16
    fp8 = mybir.dt.float8e4
    i32 = mybir.dt.int32
    seg_cols = [(s, min(512, F - s)) for s in range(0, F, 512)]

    nc = bass.Bass()
    poseT_d = nc.declare_dram_parameter("poseT", [9, Ep], bf16, isOutput=False)
    w1_d = nc.declare_dram_parameter("w1", [9, C], bf16, isOutput=False)
    b1t_d = nc.declare_dram_parameter("b1t", [P, KT], f32, isOutput=False)
    w2f8_d = nc.declare_dram_parameter("w2f8", [C, 2 * CS], fp8, isOutput=False)
    b2_d = nc.declare_dram_parameter("b2row", [1, 2 * CS], bf16, isOutput=False)
    img_d = nc.declare_dram_parameter("image", [Nn, F], bf16, isOutput=False)
    idx_d = nc.declare_dram_parameter("idx", [P, B], i32, isOutput=False)
    oh_d = nc.declare_dram_parameter("oh", [P, B * P], bf16, isOutput=False)
    recip_d = nc.declare_dram_parameter("recip", [P, n_win], f32, isOutput=False)
    out_d = nc.declare_dram_parameter("out", [Nn, F], bf16, isOutput=True)

    with TileContext(nc) as tc:
        with (
            tc.tile_pool(name="const", bufs=1) as constp,
            tc.tile_pool(name="ht", bufs=3) as htp,
            tc.tile_pool(name="gb", bufs=14) as gbp,
            tc.tile_pool(name="xg", bufs=GA + 2) as xp,
            tc.tile_pool(name="mm", bufs=14) as mp,
            tc.tile_pool(name="outp", bufs=2) as outp,
            tc.tile_pool(name="pws", bufs=3, space="PSUM") as pwsp,
            tc.tile_pool(name="ps", bufs=5, space="PSUM") as psp,
        ):
            # ---- preload constants (first chunk's pose slice first, so the
            # PE can start immediately; the rest streams behind it) ----
            w1_sb = constp.tile([41, C], bf16)
            nc.scalar.dma_start(out=w1_sb[0:9, :], in_=w1_d[:])
            nc.scalar.dma_start(out=w1_sb[32:41, :], in_=w1_d[:])
            poseT_sb = constp.tile([41, Ep], bf16)
            nc.scalar.dma_start(out=poseT_sb[0:9, :CH_EDGES], in_=poseT_d[:, :CH_EDGES])
            nc.scalar.dma_start(
                out=poseT_sb[32:41, :CH_EDGES], in_=poseT_d[:, :CH_EDGES]
            )
            nc.sync.dma_start(out=poseT_sb[0:9, CH_EDGES:], in_=poseT_d[:, CH_EDGES:])
            nc.sync.dma_start(
                out=poseT_sb[32:41, CH_EDGES:], in_=poseT_d[:, CH_EDGES:]
            )
            b1_sb = constp.tile([P, KT], f32)
            nc.sync.dma_start(out=b1_sb[:], in_=b1t_d[:])
            w2_sb = constp.tile([P, KT * 2 * CS], fp8)
            for t in range(KT):
                nc.sync.dma_start(
                    out=w2_sb[:, t * 2 * CS : (t + 1) * 2 * CS],
                    in_=w2f8_d[t * P : (t + 1) * P, :],
                )
            b2_sb = constp.tile([1, 2 * CS], bf16)
            if not meta["b2_allzero"]:
                nc.sync.dma_start(out=b2_sb[:], in_=b2_d[:])
            idx_sb = constp.tile([P, B], i32)
            nc.scalar.dma_start(out=idx_sb[:], in_=idx_d[:])
            oh_sb = constp.tile([P, B * P], bf16)
            nc.sync.dma_start(out=oh_sb[:], in_=oh_d[:])
            recip_sb = constp.tile([P, n_win], f32)
            nc.sync.dma_start(out=recip_sb[:], in_=recip_d[:])
            ones_sb = constp.tile([1, P], bf16)
            if not meta["b2_allzero"]:
                nc.gpsimd.memset(ones_sb[:], 1.0)

            w23 = w2_sb.rearrange("p (t c) -> p t c", t=KT)

            hT_tiles = {}  # chunk -> tile
            h_queue = []  # pending (ci, g, e0) h iterations, FIFO
            h_queued_ci = -1
            evac_ctr = [0]

            def queue_h(ci):
                nonlocal h_queued_ci
                while h_queued_ci < ci:
                    h_queued_ci += 1
                    if h_queued_ci >= NCH:
                        return
                    hT_tiles[h_queued_ci] = htp.tile(
                        [P, KT * CH_EDGES], fp8, tag="ht", name=f"hT{h_queued_ci}"
                    )
                    for g in range(KT // 2):
                        for e0 in range(0, CH_EDGES, 512):
                            h_queue.append((h_queued_ci, g, e0))

            def emit_h_iter(ci, g, e0):
                """one 2x row-tiled K=9 matmul pair of the hT chunk."""
                hT = hT_tiles[ci]
                e_lo = ci * CH_EDGES
                pha = psp.tile([P, 512], f32, tag="ps")
                phb = psp.tile([P, 512], f32, tag="ps")
                t0, t1 = 2 * g, 2 * g + 1
                nc.tensor.matmul(
                    out=pha[:],
                    lhsT=w1_sb[0:9, t0 * P : (t0 + 1) * P],
                    rhs=poseT_sb[0:9, e_lo + e0 : e_lo + e0 + 512],
                    start=True,
                    stop=True,
                    tile_position=(0, 0),
                )
                nc.tensor.matmul(
                    out=phb[:],
                    lhsT=w1_sb[32:41, t1 * P : (t1 + 1) * P],
                    rhs=poseT_sb[32:41, e_lo + e0 : e_lo + e0 + 512],
                    start=True,
                    stop=True,
                    tile_position=(32, 0),
                )
                for t, ph in ((t0, pha), (t1, phb)):
                    dstv = hT[:, t * CH_EDGES + e0 : t * CH_EDGES + e0 + 512]
                    evac_ctr[0] += 1
                    if evac_ctr[0] % HT_DVE_RATIO != HT_DVE_RATIO - 1:
                        nc.scalar.activation(
                            dstv,
                            ph[:],
                            mybir.ActivationFunctionType.Relu,
                            bias=b1_sb[:, t : t + 1],
                            scale=1.0,
                        )
                    elif meta["b1_allzero"]:
                        nc.vector.tensor_scalar_max(out=dstv, in0=ph[:], scalar1=0.0)
                    else:
                        nc.vector.tensor_scalar(
                            out=dstv,
                            in0=ph[:],
                            scalar1=b1_sb[:, t : t + 1],
                            scalar2=0.0,
                            op0=mybir.AluOpType.add,
                            op1=mybir.AluOpType.max,
                        )

            def pump_h(n):
                for _ in range(min(n, len(h_queue))):
                    emit_h_iter(*h_queue.pop(0))

            def flush_h(ci):
                queue_h(ci)
                while h_queue and h_queue[0][0] <= ci:
                    emit_h_iter(*h_queue.pop(0))

            def emit_gather(b):
                Xb = xp.tile([P, F], bf16, tag="xg", name=f"X{b}")
                nc.gpsimd.indirect_dma_start(
                    out=Xb[:],
                    out_offset=None,
                    in_=img_d[:],
                    in_offset=bass.IndirectOffsetOnAxis(
                        ap=idx_sb[:, b : b + 1], axis=0
                    ),
                )
                return Xb

            def emit_gb(b, ci):
                """gamma/beta for one 128-edge block: fp8 DoubleRow eT."""
                hT3 = hT_tiles[ci].rearrange("p (t e) -> p t e", t=KT)
                bi = b - ci * BPC
                pe_ps = psp.tile([P, 512], f32, tag="ps")
                for t2 in range(KT // 2):
                    nc.tensor.matmul(
                        out=pe_ps[:, : 2 * CS],
                        lhsT=hT3[:, 2 * t2 : 2 * t2 + 2, bi * P : (bi + 1) * P],
                        rhs=w23[:, 2 * t2 : 2 * t2 + 2, :],
                        start=(t2 == 0),
                        stop=(t2 == KT // 2 - 1 and meta["b2_allzero"]),
                        perf_mode=mybir.MatmulPerfMode.DoubleRow,
                    )
                if not meta["b2_allzero"]:
                    nc.tensor.matmul(
                        out=pe_ps[:, : 2 * CS],
                        lhsT=ones_sb[:1, :P],
                        rhs=b2_sb[:1, :],
                        start=False,
                        stop=True,
                    )
                return pe_ps

            # ---- HAM warm-up: dense matmuls into one scratch PSUM bank ----
            warm_ps = psp.tile([P, 512], f32, tag="ps")
            for _ in range(24):
                nc.tensor.matmul(
                    out=warm_ps[:],
                    lhsT=w1_sb[0:9, 0:P],
                    rhs=poseT_sb[0:9, 0:512],
                    start=True,
                    stop=True,
                )

            # ---- pre-issue first gathers so X data is in flight early ----
            X_tiles = {}
            for b in range(min(GA, B)):
                X_tiles[b] = emit_gather(b)

            def emit_window_scatter(w, blocks, m_tiles, gb_tiles):
                """Dense per-window PE burst, seg-major: each segment (incl the
                beta segment) accumulates over the window's blocks in its own
                single-bank PSUM tile, then evacuates immediately."""
                # beta segment first
                pbs = pwsp.tile([P, 512], f32, tag="pws", name=f"pb{w}")
                for k, b in enumerate(blocks):
                    nc.tensor.matmul(
                        out=pbs[:, :CS],
                        lhsT=oh_sb[:, b * P : (b + 1) * P],
                        rhs=gb_tiles[b][:, CS : 2 * CS],
                        start=(k == 0),
                        stop=(k == len(blocks) - 1),
                        skip_group_check=True,
                    )
                bs = outp.tile([P, CS], bf16, tag="bs", name=f"bs{w}")
                nc.scalar.activation(
                    bs[:],
                    pbs[:, :CS],
                    mybir.ActivationFunctionType.Copy,
                    scale=recip_sb[:, w : w + 1],
                )
                of = outp.tile([P, F], bf16, tag="of", name=f"of{w}")
                for si, (s, width) in enumerate(seg_cols):
                    pseg = pwsp.tile([P, 512], f32, tag="pws", name=f"pw{w}_{si}")
                    for k, b in enumerate(blocks):
                        nc.tensor.matmul(
                            out=pseg[:, :width],
                            lhsT=oh_sb[:, b * P : (b + 1) * P],
                            rhs=m_tiles[b][:, s : s + width],
                            start=(k == 0),
                            stop=(k == len(blocks) - 1),
                            skip_group_check=True,
                        )
                    # evacuate this segment: of[seg] = pseg * recip
                    if si % 2 == 0:
                        nc.scalar.activation(
                            of[:, s : s + width],
                            pseg[:, :width],
                            mybir.ActivationFunctionType.Copy,
                            scale=recip_sb[:, w : w + 1],
                        )
                    else:
                        nc.vector.tensor_scalar_mul(
                            out=of[:, s : s + width],
                            in0=pseg[:, :width],
                            scalar1=recip_sb[:, w : w + 1],
                        )
                    pump_h(1)
                # add the per-window beta (broadcast over hw) in one DVE pass
                bs_b = bs.rearrange("p (o c) -> p o c", o=1)
                of3 = of.rearrange("p (o c) -> p o c", o=HW)
                nc.vector.tensor_tensor(
                    out=of3,
                    in0=of3,
                    in1=bs_b.to_broadcast([P, HW, CS]),
                    op=mybir.AluOpType.add,
                )
                nc.sync.dma_start(out=out_d[w * P : (w + 1) * P, :], in_=of[:])

            # ---- main pipeline over windows/blocks ----
            next_gather = min(GA, B)
            m_tiles = {}
            gb_tiles = {}
            pending_win = []

            for w in range(n_win):
                blocks = win_blocks[w]
                for k, b in enumerate(blocks):
                    # emit the previous window's scatter burst 2 blocks into
                    # this window, so its last m-tiles have DVE slack
                    if k == 2 and pending_win:
                        wd, blksd = pending_win.pop(0)
                        emit_window_scatter(wd, blksd, m_tiles, gb_tiles)
                        for bd in blksd:
                            del m_tiles[bd], gb_tiles[bd]

                    ci = b // BPC
                    flush_h(ci)
                    queue_h(ci + 1)
                    if next_gather < B:
                        X_tiles[next_gather] = emit_gather(next_gather)
                        next_gather += 1

                    pe_ps = emit_gb(b, ci)
                    gbt = gbp.tile([P, 2 * CS], bf16, tag="gb", name=f"gb{b}")
                    gb_tiles[b] = gbt
                    nc.scalar.activation(
                        gbt[:],
                        pe_ps[:, : 2 * CS],
                        mybir.ActivationFunctionType.Sigmoid,
                    )
                    pump_h(1)

                    mt = mp.tile([P, F], bf16, tag="mm", name=f"m{b}")
                    m_tiles[b] = mt
                    Xb = X_tiles.pop(b)
                    g_b = (
                        gbt[:, :CS]
                        .rearrange("p (o c) -> p o c", o=1)
                        .to_broadcast([P, HW, CS])
                    )
                    nc.vector.tensor_tensor(
                        out=mt.rearrange("p (o c) -> p o c", o=HW),
                        in0=Xb.rearrange("p (o c) -> p o c", o=HW),
                        in1=g_b,
                        op=mybir.AluOpType.mult,
                    )
                    pump_h(1)

                pending_win.append((w, blocks))

            while pending_win:
                wd, blksd = pending_win.pop(0)
                emit_window_scatter(wd, blksd, m_tiles, gb_tiles)
                for bd in blksd:
                    del m_tiles[bd], gb_tiles[bd]

    _split_excess_waits(nc)
    return nc


def _run(inputs, trace=False, trace_kwargs=None):
    pose = np.asarray(inputs["pose"], np.float32)
    image = np.asarray(inputs["image"], np.float32)
    W1 = np.asarray(inputs["W1"], np.float32)
    b1 = np.asarray(inputs["b1"], np.float32)
    W2 = np.asarray(inputs["W2"], np.float32)
    b2 = np.asarray(inputs["b2"], np.float32)
    src = np.asarray(inputs["src"])
    dst = np.asarray(inputs["dst"])

    in_maps, meta = _host_prep(pose, image, W1, b1, W2, b2, src, dst)
    nc = _build(meta)
    kw = {}
    if trace:
        kw = dict(trace=True, trace_kwargs=trace_kwargs or {})
    res = run_bass_kernel_spmd(nc, in_maps, core_ids=list(range(N_CORES)), **kw)
    Nn, C, HW, CS = meta["Nn"], meta["C"], meta["HW"], meta["CS"]
    H = int(np.sqrt(HW))
    out = np.empty((Nn, C, H, HW // H), np.float32)
    for j in range(N_CORES):
        oc = np.asarray(res.results[j]["out"]).astype(np.float32)
        out[:, j * CS : (j + 1) * CS] = (
            oc.reshape(Nn, HW, CS).transpose(0, 2, 1).reshape(Nn, CS, H, HW // H)
        )
    return out, res


def kernel(**inputs) -> np.ndarray:
    out, _ = _run(inputs)
    return out
